# revision 1
# baseline (speedup 1.0000x reference)
"""Trainium2 Bass kernel for nn_CrossAttention (B=4, Lq=Lk=E=1024, H=16).

Sharding: data-parallel over 8 cores; core c handles batch c//2, query rows
(c%2)*512 ... +512. Heads stay local to a core, so softmax + head-mean need
no collectives. Each core computes a [512,1024] slice of attn_output and
attn_scores.

Per-core pipeline:
  rmsnorm (row layout, DVE sumsq + ACT sqrt + DVE recip)
  -> PE transpose of normalized x to [e, i] layout
  -> q/k projections as W_eff.T @ xnT on PE (gains and 1/sqrt(hd) folded into
     weights on the host), f32r matmuls
  -> per head: logits = qT_h.T @ kT_h into PSUM; ACT Exp with fused row-sum
     (accum_out); head accumulation of w_h * E_h on the PE via a diag(w_h)
     stationary matmul accumulating into a PSUM scores tile
  -> scores evacuated + PE-transposed; attn_output = scoresT.T @ V on PE.
"""

import numpy as np
from contextlib import ExitStack

B, LQ, LK, E = 4, 1024, 1024, 1024
H = 16
HD = E // H  # 64
N_CORES = 8
QROWS = LQ // 2  # 512 rows of q per core
EPS = 1.1920929e-07

_CACHE = {}


def _build_program():
    import concourse.bass as bass
    import concourse.tile as tile
    from concourse import bacc, mybir

    f32 = mybir.dt.float32
    f32r = mybir.dt.float32r
    Alu = mybir.AluOpType
    Act = mybir.ActivationFunctionType

    nc = bacc.Bacc("TRN2", target_bir_lowering=False, debug=False,
                   num_devices=N_CORES)

    xq = nc.dram_tensor("xq", [QROWS, E], f32, kind="ExternalInput").ap()
    xk = nc.dram_tensor("xk", [LK, E], f32, kind="ExternalInput").ap()
    vv = nc.dram_tensor("vv", [LK, E], f32r, kind="ExternalInput").ap()
    wqt = nc.dram_tensor("wqt", [E, E], f32r, kind="ExternalInput").ap()
    wkt = nc.dram_tensor("wkt", [E, E], f32r, kind="ExternalInput").ap()
    bq = nc.dram_tensor("bq", [E, 1], f32, kind="ExternalInput").ap()
    bk = nc.dram_tensor("bk", [E, 1], f32, kind="ExternalInput").ap()
    ident = nc.dram_tensor("ident", [128, 128], f32r, kind="ExternalInput").ap()

    out = nc.dram_tensor("out", [QROWS, E], f32, kind="ExternalOutput").ap()
    sc = nc.dram_tensor("sc", [QROWS, LK], f32, kind="ExternalOutput").ap()

    def r(ap):
        return ap

    with tile.TileContext(nc) as tc, ExitStack() as ctx:
        const_pool = ctx.enter_context(tc.tile_pool(name="const", bufs=1))
        id_sb = const_pool.tile([128, 128], f32r)
        nc.sync.dma_start(id_sb[:], ident[:])
        eps_sb = const_pool.tile([128, 1], f32, name="eps_sb")
        nc.vector.memset(eps_sb[:], EPS)
        bq_sb = const_pool.tile([128, 8], f32, name="bq_sb")
        bk_sb = const_pool.tile([128, 8], f32, name="bk_sb")
        for o in range(8):
            nc.sync.dma_start(bq_sb[:, o : o + 1], bq[o * 128 : (o + 1) * 128, :])
            nc.sync.dma_start(bk_sb[:, o : o + 1], bk[o * 128 : (o + 1) * 128, :])

        stats = ctx.enter_context(tc.tile_pool(name="stats", bufs=6))
        ps_small = ctx.enter_context(
            tc.tile_pool(name="ps_small", bufs=2, space="PSUM")
        )
        ps_big = ctx.enter_context(tc.tile_pool(name="ps_big", bufs=2, space="PSUM"))

        kT_pool = ctx.enter_context(tc.tile_pool(name="kT", bufs=1))
        qT_pool = ctx.enter_context(tc.tile_pool(name="qT", bufs=1))
        v_pool = ctx.enter_context(tc.tile_pool(name="vsb", bufs=1))
        kT = [kT_pool.tile([128, LK], f32r, name=f"kT{o}") for o in range(8)]
        qT = [qT_pool.tile([128, QROWS], f32r, name=f"qT{o}") for o in range(8)]
        v_sb = [v_pool.tile([128, E], f32r, name=f"v{j}") for j in range(8)]

        # ---- rmsnorm + transpose + projection for q and k --------------------
        with ExitStack() as pctx:
            xstage = pctx.enter_context(tc.tile_pool(name="xstage", bufs=3))
            xn_pool = pctx.enter_context(tc.tile_pool(name="xn", bufs=2))
            xT_pool = pctx.enter_context(tc.tile_pool(name="xT", bufs=1))
            w_pool = pctx.enter_context(tc.tile_pool(name="wst", bufs=1))

            xkT = [xT_pool.tile([128, LK], f32r, name=f"xkT{e}", tag=f"xkT{e}") for e in range(8)]
            xqT = [xT_pool.tile([128, QROWS], f32r, name=f"xqT{e}", tag=f"xqT{e}") for e in range(8)]

            def norm_transpose(x_dram, nrows_tiles, xT_tiles):
                for t in range(nrows_tiles):
                    x = xstage.tile([128, E], f32, tag="xstage")
                    nc.sync.dma_start(x[:], x_dram[t * 128 : (t + 1) * 128, :])
                    xn = xn_pool.tile([128, E], f32r, tag="xn")
                    sqscr = xn_pool.tile([128, E], f32, tag="sqscr")
                    ssq = stats.tile([128, 1], f32, tag="stats")
                    nc.scalar.activation(sqscr[:], x[:], Act.Square,
                                         accum_out=ssq[:])
                    s = stats.tile([128, 1], f32, tag="stats")
                    nc.scalar.activation(s[:], ssq[:], Act.Sqrt,
                                         bias=eps_sb[:], scale=1.0 / E)
                    inv = stats.tile([128, 1], f32, tag="stats")
                    nc.vector.reciprocal(inv[:], s[:])
                    nc.vector.tensor_scalar(
                        out=xn[:], in0=x[:], scalar1=inv[:], scalar2=None,
                        op0=Alu.mult,
                    )
                    for eb in range(8):
                        tp = ps_small.tile([128, 128], f32r, tag="tp")
                        nc.tensor.transpose(
                            tp[:], xn[:, eb * 128 : (eb + 1) * 128], id_sb[:]
                        )
                        nc.vector.tensor_copy(
                            xT_tiles[eb][:, t * 128 : (t + 1) * 128], tp[:]
                        )

            norm_transpose(xk, 8, xkT)
            norm_transpose(xq, 4, xqT)

            wk_sb = [w_pool.tile([128, E], f32r, name=f"wk{e}", tag=f"w{e}") for e in range(8)]
            for e in range(8):
                nc.sync.dma_start(wk_sb[e][:], wkt[e * 128 : (e + 1) * 128, :])
            for o in range(8):
                pk = ps_big.tile([128, LK], f32, tag="psbig")
                for half in range(2):
                    cols = slice(half * 512, half * 512 + 512)
                    for e in range(8):
                        nc.tensor.matmul(
                            pk[:, cols],
                            lhsT=r(wk_sb[e][:, o * 128 : (o + 1) * 128]),
                            rhs=r(xkT[e][:, cols]),
                            start=(e == 0), stop=(e == 7),
                        )
                nc.vector.tensor_scalar(
                    out=kT[o][:], in0=pk[:], scalar1=bk_sb[:, o : o + 1],
                    scalar2=None, op0=Alu.add,
                )

            wq_sb = [w_pool.tile([128, E], f32r, name=f"wq{e}", tag=f"w{e}") for e in range(8)]
            for e in range(8):
                nc.sync.dma_start(wq_sb[e][:], wqt[e * 128 : (e + 1) * 128, :])
            for o in range(8):
                pq = ps_big.tile([128, QROWS], f32, tag="psbig")
                for e in range(8):
                    nc.tensor.matmul(
                        pq[:],
                        lhsT=r(wq_sb[e][:, o * 128 : (o + 1) * 128]),
                        rhs=r(xqT[e][:]),
                        start=(e == 0), stop=(e == 7),
                    )
                nc.vector.tensor_scalar(
                    out=qT[o][:], in0=pq[:], scalar1=bq_sb[:, o : o + 1],
                    scalar2=None, op0=Alu.add,
                )

        for j in range(8):
            nc.sync.dma_start(v_sb[j][:], vv[j * 128 : (j + 1) * 128, :])

        # ---- attention ------------------------------------------------------
        with ExitStack() as actx:
            ps_sc = actx.enter_context(
                tc.tile_pool(name="ps_sc", bufs=1, space="PSUM")
            )
            e_pool = actx.enter_context(tc.tile_pool(name="epool", bufs=3))
            diag_pool = actx.enter_context(tc.tile_pool(name="diag", bufs=3))
            scs_pool = actx.enter_context(tc.tile_pool(name="scs", bufs=2))
            scT_pool = actx.enter_context(tc.tile_pool(name="scT", bufs=2))
            osb_pool = actx.enter_context(tc.tile_pool(name="osb", bufs=3))

            for it in range(4):
                icols = slice(it * 128, (it + 1) * 128)
                sp = ps_sc.tile([128, LK], f32, tag="spsum")
                for h in range(H):
                    oc, po = h // 2, (h % 2) * 64
                    lg = ps_big.tile([128, LK], f32, tag="psbig")
                    lhs_q = qT[oc][po : po + 64, icols]
                    for half in range(2):
                        cols = slice(half * 512, half * 512 + 512)
                        nc.tensor.matmul(
                            lg[:, cols], lhsT=r(lhs_q), rhs=r(kT[oc][po : po + 64, cols]),
                            start=True, stop=True,
                        )
                    Et = e_pool.tile([128, LK], f32r, tag="E")
                    ssum = stats.tile([128, 1], f32, tag="stats")
                    nc.scalar.activation(Et[:], lg[:], Act.Exp, accum_out=ssum[:])
                    w = stats.tile([128, 1], f32, tag="stats")
                    nc.vector.reciprocal(w[:], ssum[:])
                    dg = diag_pool.tile([128, 128], f32r, tag="diag")
                    nc.vector.tensor_scalar(
                        out=dg[:], in0=id_sb[:], scalar1=w[:], scalar2=1.0 / H,
                        op0=Alu.mult, op1=Alu.mult,
                    )
                    for half in range(2):
                        cols = slice(half * 512, half * 512 + 512)
                        nc.tensor.matmul(
                            sp[:, cols], lhsT=r(dg[:]), rhs=r(Et[:, cols]),
                            start=(h == 0), stop=(h == H - 1),
                        )
                scs = scs_pool.tile([128, LK], f32r, tag="scs")
                nc.vector.tensor_copy(scs[:], sp[:])
                nc.sync.dma_start(sc[icols, :], scs[:].bitcast(f32))
                scT = [scT_pool.tile([128, 128], f32r, name=f"scT_{it}_{j}", tag=f"scT{j}")
                       for j in range(8)]
                for j in range(8):
                    tp = ps_small.tile([128, 128], f32r, tag="tp")
                    nc.tensor.transpose(
                        tp[:], scs[:, j * 128 : (j + 1) * 128], id_sb[:]
                    )
                    nc.vector.tensor_copy(scT[j][:], tp[:])
                for half in range(2):
                    cols = slice(half * 512, half * 512 + 512)
                    op = ps_small.tile([128, 512], f32, tag="tp")
                    for j in range(8):
                        nc.tensor.matmul(
                            op[:], lhsT=r(scT[j][:]), rhs=r(v_sb[j][:, cols]),
                            start=(j == 0), stop=(j == 7),
                        )
                    osb = osb_pool.tile([128, 512], f32, tag="osb")
                    nc.vector.tensor_copy(osb[:], op[:])
                    nc.sync.dma_start(out[icols, cols], osb[:])

    nc.compile()
    return nc


def _get_program():
    if "nc" not in _CACHE:
        _CACHE["nc"] = _build_program()
    return _CACHE["nc"]


def kernel(query, key, value, gq, gk, Wq, bq, Wk, bk):
    from concourse.bass_utils import run_bass_kernel_spmd

    nc = _get_program()

    scale = 1.0 / np.sqrt(np.float32(HD))
    wqt = np.ascontiguousarray((Wq * gq[None, :] * scale).T, dtype=np.float32)
    wkt = np.ascontiguousarray((Wk * gk[None, :]).T, dtype=np.float32)
    bq2 = np.ascontiguousarray((bq * scale).reshape(E, 1), dtype=np.float32)
    bk2 = np.ascontiguousarray(bk.reshape(E, 1), dtype=np.float32)
    ident = np.eye(128, dtype=np.float32)

    in_maps = []
    for c in range(N_CORES):
        b, half = divmod(c, 2)
        i0 = half * QROWS
        in_maps.append({
            "xq": np.ascontiguousarray(query[b, i0 : i0 + QROWS], dtype=np.float32),
            "xk": np.ascontiguousarray(key[b], dtype=np.float32),
            "vv": np.ascontiguousarray(value[b], dtype=np.float32),
            "wqt": wqt, "wkt": wkt, "bq": bq2, "bk": bk2, "ident": ident,
        })

    res = run_bass_kernel_spmd(nc, in_maps, list(range(N_CORES)))

    attn_output = np.empty((B, LQ, E), dtype=np.float32)
    attn_scores = np.empty((B, LQ, LK), dtype=np.float32)
    for c in range(N_CORES):
        b, half = divmod(c, 2)
        i0 = half * QROWS
        attn_output[b, i0 : i0 + QROWS] = res.results[c]["out"]
        attn_scores[b, i0 : i0 + QROWS] = res.results[c]["sc"]
    return attn_output, attn_scores



# revision 7
# speedup vs baseline: 1.2018x; 1.2018x over previous
"""Trainium2 Bass kernel for nn_CrossAttention (B=4, Lq=Lk=E=1024, H=16).

Sharding: data-parallel over 8 cores; core c handles batch c//2, query rows
(c%2)*512 ... +512. Heads stay local to a core, so softmax + head-mean need
no collectives.

v2 pipeline (per core):
  rmsnorm via ACT Square(+accum)/Sqrt and a DVE divide, PE transposes of the
  normalized x into token-major xT layouts
  -> q/k projections as W_eff.T @ xT on PE (gains and 1/sqrt(hd) folded into
     the weights on the host), f32r matmuls, per-o weight tiles streamed from
     DRAM in o-major layout
  -> per (it, head): logits = qT_h.T @ kT_h into PSUM (f32r, free=512);
     ACT Exp -> E bf16; DVE tensor_scalar copy with accum_out -> row sums S;
     DVE divide builds dg = diag(1/S) bf16; 8 bf16 PE matmuls
     (lhsT=E-slice, rhs=dg) accumulate the *transposed* head-mean scores
     scT[j, i] directly in PSUM across all 16 heads — softmax normalization,
     head averaging (1/H folded into V on the host) and the transpose needed
     by the output matmul all happen inside the PSUM accumulation.
  -> scT evacuated once per it as bf16; attn_output = scT.T @ (V/H) on PE in
     bf16; outputs DMA'd as bf16, host transposes scores / upcasts.
"""

import numpy as np
from contextlib import ExitStack

B, LQ, LK, E = 4, 1024, 1024, 1024
H = 16
HD = E // H  # 64
N_CORES = 8
QROWS = LQ // 2  # 512 rows of q per core
EPS = 1.1920929e-07

_CACHE = {}


def _build_program():
    import concourse.bass as bass
    import concourse.tile as tile
    from concourse import bacc, mybir

    f32 = mybir.dt.float32
    f32r = mybir.dt.float32r
    bf16 = mybir.dt.bfloat16
    Alu = mybir.AluOpType
    Act = mybir.ActivationFunctionType
    AxX = mybir.AxisListType.X

    nc = bacc.Bacc("TRN2", target_bir_lowering=False, debug=False,
                   num_devices=N_CORES)

    xq = nc.dram_tensor("xq", [QROWS, E], f32, kind="ExternalInput").ap()
    xk = nc.dram_tensor("xk", [LK, E], f32, kind="ExternalInput").ap()
    vv = nc.dram_tensor("vv", [LK, E], bf16, kind="ExternalInput").ap()
    wk = nc.dram_tensor("wk", [E, E], f32r, kind="ExternalInput").ap()
    wq = nc.dram_tensor("wq", [E, E], f32r, kind="ExternalInput").ap()
    bqv = nc.dram_tensor("bqv", [128, 8], f32, kind="ExternalInput").ap()
    bkv = nc.dram_tensor("bkv", [128, 8], f32, kind="ExternalInput").ap()
    idf = nc.dram_tensor("idf", [128, 128], f32r, kind="ExternalInput").ap()
    idb = nc.dram_tensor("idb", [128, 128], bf16, kind="ExternalInput").ap()

    outb = nc.dram_tensor("outb", [QROWS, E], bf16, kind="ExternalOutput").ap()
    scT = nc.dram_tensor("scT", [LK, QROWS], bf16, kind="ExternalOutput").ap()

    with tile.TileContext(nc) as tc, ExitStack() as ctx:
        const_pool = ctx.enter_context(tc.tile_pool(name="const", bufs=1))
        id_f = const_pool.tile([128, 128], f32r)
        id_b = const_pool.tile([128, 128], bf16)
        bq_sb = const_pool.tile([128, 8], f32)
        bk_sb = const_pool.tile([128, 8], f32)
        eps_sb = const_pool.tile([128, 1], f32)
        nc.sync.dma_start(id_f[:], idf[:])
        nc.sync.dma_start(id_b[:], idb[:])
        nc.sync.dma_start(bq_sb[:], bqv[:])
        nc.sync.dma_start(bk_sb[:], bkv[:])
        nc.vector.memset(eps_sb[:], EPS)

        big = ctx.enter_context(tc.tile_pool(name="big", bufs=1))
        # token-major transposed normalized inputs: col = tok_tile*E + e*128+ep
        xkT = big.tile([128, 8 * LK], f32r)
        xqT = big.tile([128, 4 * LK], f32r)
        # o-major projected tensors: col = o*LK + tok (kT) / o*QROWS + tok (qT)
        kT = big.tile([128, 8 * LK], f32r)
        qT = big.tile([128, 8 * QROWS], f32r)
        vbig = big.tile([128, 8 * LK], bf16)  # col = jt*E + d

        stats = ctx.enter_context(tc.tile_pool(name="stats", bufs=1))
        ssq = stats.tile([128, 12], f32)
        srt = stats.tile([128, 12], f32)
        sinv = stats.tile([128, 12], f32)
        ssum_pool = ctx.enter_context(tc.tile_pool(name="ssum", bufs=6))

        # 3D views for proj rhs: [p, tok_tile, col-within-tile]
        xkT3 = xkT[:].rearrange("p (t c) -> p t c", t=8)
        xqT3 = xqT[:].rearrange("p (t c) -> p t c", t=4)

        with ExitStack() as pctx:
            xst = pctx.enter_context(tc.tile_pool(name="xst", bufs=3))
            xqst = pctx.enter_context(tc.tile_pool(name="xqst", bufs=2))
            scr_pool = pctx.enter_context(tc.tile_pool(name="scr", bufs=2))
            xn_pool = pctx.enter_context(tc.tile_pool(name="xn", bufs=2))
            tp_pool = pctx.enter_context(
                tc.tile_pool(name="tp", bufs=2, space="PSUM"))
            pk_pool = pctx.enter_context(
                tc.tile_pool(name="pk", bufs=2, space="PSUM"))
            wk_pool = pctx.enter_context(tc.tile_pool(name="wkp", bufs=3))
            wq_pool = pctx.enter_context(tc.tile_pool(name="wqp", bufs=3))

            # ---- DMA queue: xk tiles, wk tiles, xq tiles, wq tiles, vv ----
            xk_t, xq_t, wk_t, wq_t = [], [], [], []
            for t in range(8):
                x = xst.tile([128, E], f32, tag="xst")
                nc.sync.dma_start(x[:], xk[t * 128:(t + 1) * 128, :])
                xk_t.append(x)
            for o in range(8):
                w = wk_pool.tile([128, E], f32r, tag="wk")
                nc.sync.dma_start(w[:], wk[o * 128:(o + 1) * 128, :])
                wk_t.append(w)
            for t in range(4):
                x = xqst.tile([128, E], f32, tag="xqst")
                nc.sync.dma_start(x[:], xq[t * 128:(t + 1) * 128, :])
                xq_t.append(x)
            for o in range(8):
                w = wq_pool.tile([128, E], f32r, tag="wq")
                nc.sync.dma_start(w[:], wq[o * 128:(o + 1) * 128, :])
                wq_t.append(w)
            for jt in range(8):
                nc.sync.dma_start(vbig[:, jt * E:(jt + 1) * E],
                                  vv[jt * 128:(jt + 1) * 128, :])

            def norm_transpose(x, col, xT_dst):
                # rmsnorm row-scale then PE-transpose into token-major layout
                scr = scr_pool.tile([128, E], f32, tag="scr")
                nc.scalar.activation(scr[:], x[:], Act.Square,
                                     accum_out=ssq[:, col:col + 1])
                nc.scalar.activation(srt[:, col:col + 1], ssq[:, col:col + 1],
                                     Act.Sqrt, bias=eps_sb[:], scale=1.0 / E)
                xn = xn_pool.tile([128, E], f32r, tag="xn")
                nc.vector.reciprocal(sinv[:, col:col + 1], srt[:, col:col + 1])
                nc.vector.tensor_scalar(out=xn[:], in0=x[:],
                                        scalar1=sinv[:, col:col + 1],
                                        scalar2=None, op0=Alu.mult)
                for g in range(2):
                    tp = tp_pool.tile([128, 512], f32r, tag="tp")
                    for b in range(4):
                        eb = g * 4 + b
                        nc.tensor.transpose(
                            tp[:, b * 128:(b + 1) * 128],
                            xn[:, eb * 128:(eb + 1) * 128], id_f[:])
                    nc.vector.tensor_copy(
                        xT_dst[:, g * 512:(g + 1) * 512], tp[:])

            for t in range(8):
                norm_transpose(xk_t[t], t, xkT[:, t * E:(t + 1) * E])

            # ---- K projection: kT[o*LK + tok] = wk_o.T @ xkT + bk ----
            for o in range(8):
                pk = pk_pool.tile([128, LK], f32, tag="pk")
                for half in range(2):
                    dst = pk[:, half * 512:(half + 1) * 512]
                    for e in range(8):
                        nc.tensor.matmul(
                            dst,
                            lhsT=wk_t[o][:, e * 128:(e + 1) * 128],
                            rhs=xkT3[:, half * 4:(half + 1) * 4,
                                     e * 128:(e + 1) * 128],
                            start=(e == 0), stop=(e == 7),
                        )
                nc.vector.tensor_scalar(
                    out=kT[:, o * LK:(o + 1) * LK], in0=pk[:],
                    scalar1=bk_sb[:, o:o + 1], scalar2=None, op0=Alu.add)

            for t in range(4):
                norm_transpose(xq_t[t], 8 + t, xqT[:, t * E:(t + 1) * E])

            # ---- Q projection ----
            for o in range(8):
                pq = pk_pool.tile([128, LK], f32, tag="pk")
                for e in range(8):
                    nc.tensor.matmul(
                        pq[:, :QROWS],
                        lhsT=wq_t[o][:, e * 128:(e + 1) * 128],
                        rhs=xqT3[:, :, e * 128:(e + 1) * 128],
                        start=(e == 0), stop=(e == 7),
                    )
                nc.vector.tensor_scalar(
                    out=qT[:, o * QROWS:(o + 1) * QROWS], in0=pq[:, :QROWS],
                    scalar1=bq_sb[:, o:o + 1], scalar2=None, op0=Alu.add)

        # ---- attention ------------------------------------------------------
        # PSUM accumulation groups cannot interleave members on HW, so the
        # scores-transpose accumulation runs jt-outer over 16 resident E/dg
        # tiles; jt-groups of it are interleaved with logits/exp of it+1 to
        # keep ACT busy.
        with ExitStack() as actx:
            lg_pool = actx.enter_context(
                tc.tile_pool(name="lg", bufs=2, space="PSUM"))
            sp_pool = actx.enter_context(
                tc.tile_pool(name="sp", bufs=2, space="PSUM"))
            e_pool = actx.enter_context(tc.tile_pool(name="epool", bufs=1))
            e2_pool = actx.enter_context(tc.tile_pool(name="e2pool", bufs=2))
            dg_pool = actx.enter_context(tc.tile_pool(name="dgp", bufs=1))
            scs_pool = actx.enter_context(tc.tile_pool(name="scs", bufs=2))
            osb_pool = actx.enter_context(tc.tile_pool(name="osb", bufs=2))

            Et = [[e_pool.tile([128, LK], bf16, name=f"E_{p}_{h}")
                   for h in range(H)] for p in range(2)]
            dgt = [[dg_pool.tile([128, 128], bf16, name=f"dg_{p}_{h}")
                    for h in range(H)] for p in range(2)]
            sp_t = [None] * 4

            def emit_head(it, h):
                p = it % 2
                oc, po = h // 2, (h % 2) * 64
                lg = lg_pool.tile([128, LK], f32, tag="lg")
                qs = qT[po:po + 64,
                        oc * QROWS + it * 128:oc * QROWS + (it + 1) * 128]
                for half in range(2):
                    nc.tensor.matmul(
                        lg[:, half * 512:(half + 1) * 512],
                        lhsT=qs,
                        rhs=kT[po:po + 64,
                               oc * LK + half * 512:oc * LK + (half + 1) * 512],
                        start=True, stop=True,
                    )
                nc.scalar.activation(Et[p][h][:], lg[:], Act.Exp)
                E2 = e2_pool.tile([128, LK], bf16, tag="E2")
                ssum = ssum_pool.tile([128, 1], f32, tag="ss")
                nc.vector.tensor_scalar(out=E2[:], in0=Et[p][h][:], scalar1=1.0,
                                        scalar2=0.0, op0=Alu.mult,
                                        op1=Alu.add, accum_out=ssum[:])
                w = ssum_pool.tile([128, 1], f32, tag="w")
                nc.vector.reciprocal(w[:], ssum[:])
                nc.vector.tensor_scalar(out=dgt[p][h][:], in0=id_b[:],
                                        scalar1=w[:], scalar2=None,
                                        op0=Alu.mult)

            def emit_scT_group(it, jt):
                p = it % 2
                for h in range(H):
                    nc.tensor.matmul(
                        sp_t[it][:, jt * 128:(jt + 1) * 128],
                        lhsT=Et[p][h][:, jt * 128:(jt + 1) * 128],
                        rhs=dgt[p][h][:],
                        start=(h == 0), stop=(h == H - 1),
                    )

            def emit_tail(it):
                scs = scs_pool.tile([128, LK], bf16, tag="scs")
                nc.vector.tensor_copy(scs[:], sp_t[it][:])
                for jt in range(8):
                    nc.sync.dma_start(
                        scT[jt * 128:(jt + 1) * 128, it * 128:(it + 1) * 128],
                        scs[:, jt * 128:(jt + 1) * 128])
                op = lg_pool.tile([128, LK], f32, tag="lg")
                for half in range(2):
                    for jt in range(8):
                        nc.tensor.matmul(
                            op[:, half * 512:(half + 1) * 512],
                            lhsT=scs[:, jt * 128:(jt + 1) * 128],
                            rhs=vbig[:, jt * E + half * 512:
                                     jt * E + (half + 1) * 512],
                            start=(jt == 0), stop=(jt == 7),
                        )
                osb = osb_pool.tile([128, E], bf16, tag="osb")
                nc.vector.tensor_copy(osb[:], op[:])
                nc.sync.dma_start(outb[it * 128:(it + 1) * 128, :], osb[:])

            for h in range(H):
                emit_head(0, h)
            for it in range(4):
                sp_t[it] = sp_pool.tile([128, LK], f32, tag="sp",
                                        name=f"sp_{it}")
                if it < 3:
                    for jt in range(8):
                        emit_scT_group(it, jt)
                        emit_head(it + 1, 2 * jt)
                        emit_head(it + 1, 2 * jt + 1)
                else:
                    for jt in range(8):
                        emit_scT_group(it, jt)
                emit_tail(it)

    nc.compile()
    return nc


def _get_program():
    if "nc" not in _CACHE:
        _CACHE["nc"] = _build_program()
    return _CACHE["nc"]


def kernel(query, key, value, gq, gk, Wq, bq, Wk, bk):
    import ml_dtypes
    from concourse.bass_utils import run_bass_kernel_spmd

    nc = _get_program()
    bf = ml_dtypes.bfloat16

    scale = 1.0 / np.sqrt(np.float32(HD))
    # W_eff.T: wqt[e, o] = (Wq[o, e] * gq[e] * scale)
    wqt = np.ascontiguousarray((np.asarray(Wq) * np.asarray(gq)[None, :]
                                * scale).T, dtype=np.float32)
    wkt = np.ascontiguousarray((np.asarray(Wk) * np.asarray(gk)[None, :]).T,
                               dtype=np.float32)
    # o-major packed weights: w_pk[o*128+ep, e*128+oc] = wt[e*128+ep, o*128+oc]
    def pack(wt):
        w4 = wt.reshape(8, 128, 8, 128)          # [e, ep, o, oc]
        return np.ascontiguousarray(
            w4.transpose(2, 1, 0, 3).reshape(E, E), dtype=np.float32)

    wq_pk = pack(wqt)
    wk_pk = pack(wkt)
    bq_pk = np.ascontiguousarray(
        (np.asarray(bq) * scale).reshape(8, 128).T, dtype=np.float32)
    bk_pk = np.ascontiguousarray(
        np.asarray(bk).reshape(8, 128).T, dtype=np.float32)
    ident = np.eye(128, dtype=np.float32)
    ident_b = np.eye(128).astype(bf)

    in_maps = []
    for c in range(N_CORES):
        b, half = divmod(c, 2)
        i0 = half * QROWS
        in_maps.append({
            "xq": np.ascontiguousarray(query[b, i0:i0 + QROWS],
                                       dtype=np.float32),
            "xk": np.ascontiguousarray(key[b], dtype=np.float32),
            "vv": np.ascontiguousarray(
                (np.asarray(value[b], dtype=np.float32) / H)).astype(bf),
            "wq": wq_pk, "wk": wk_pk, "bqv": bq_pk, "bkv": bk_pk,
            "idf": ident, "idb": ident_b,
        })

    res = run_bass_kernel_spmd(nc, in_maps, list(range(N_CORES)))

    attn_output = np.empty((B, LQ, E), dtype=np.float32)
    attn_scores = np.empty((B, LQ, LK), dtype=np.float32)
    for c in range(N_CORES):
        b, half = divmod(c, 2)
        i0 = half * QROWS
        attn_output[b, i0:i0 + QROWS] = np.asarray(
            res.results[c]["outb"], dtype=np.float32)
        # scT is [LK, QROWS] = H * scores^T
        attn_scores[b, i0:i0 + QROWS] = (
            np.asarray(res.results[c]["scT"], dtype=np.float32).T / H)
    return attn_output, attn_scores
